# revision 6
# baseline (speedup 1.0000x reference)
"""Kernel for nn_DSRB: spiking dense-CNN block on 8 Trainium NeuronCores.

The axon tunnel to the devices moves ~50 MB/s, so wall time is dominated by
host<->device bytes, not FLOPs. Design:

- x only ever feeds lif(x), whose binary spikes are identical at every
  layer, so the host computes lif(x) and ships bit-packed spikes
  (2.4 MB instead of the 67 MB fp32 x), overlapped with the uploads.
- Sharding: (batch b, H-half) -> 8 cores. Each core carries 5 redundant
  margin rows through the conv stack instead of exchanging halos; BN
  stats are psum'd; per-batch attention means use a scatter+psum trick.
- Activations are channel-first [C,T,rows,W] so every conv einsum is a
  direct [o,i]x[i,t*h*w] matmul with no layout transposes (this cut
  device exec from ~200 ms to ~20 ms).
- Output is the attention term only, int8 with a fixed scale (16.7 MB);
  the x residual is added back on host during the threaded fetch.
"""

import os
import time
import hashlib
import numpy as np
import jax
import jax.numpy as jnp
from concurrent.futures import ThreadPoolExecutor, as_completed

TAU = 2.0
VTH = 0.15
EPS = 1e-5

T, B, C, H, W = 4, 4, 64, 128, 128
GR = 24
M = 5
R = 64 + 2 * M
ND = 8
QSCALE = 0.625 / 127.0

_WNAMES = ('w0', 'w1', 'w2', 'w3', 'g0', 'g1', 'g2', 'g3',
           'b0', 'b1', 'b2', 'b3', 'lff_w', 't_w', 't_b',
           'c_w1', 'c_b1', 'c_w2', 'c_b2', 's_w', 's_b')

_cache = {'key': None, 'fn': None}
_PROF = bool(os.environ.get('KPROF'))


def _conv3(s, w):
    # s: [Ci, T, rows, 128], w: [Co, Ci, 3, 3]; f32 accumulation.
    rows = s.shape[2]
    xp = jnp.pad(s, ((0, 0), (0, 0), (1, 1), (1, 1)))
    acc = None
    for dy in range(3):
        for dx in range(3):
            xs = xp[:, :, dy:dy + rows, dx:dx + W]
            t = jnp.einsum('oi,ithw->othw', w[:, :, dy, dx], xs,
                           preferred_element_type=jnp.float32)
            acc = t if acc is None else acc + t
    return acc


def _lif4(y):
    # y: [c, T, rows, 128] f32 -> bf16 spikes, same layout.
    v = jnp.zeros_like(y[:, 0])
    outs = []
    for t in range(T):
        v = v * 0.5 + y[:, t]
        s = (v >= VTH).astype(y.dtype)
        outs.append(s)
        v = v * (1.0 - s)
    return jnp.stack(outs, axis=1)


def _make_fn(wc):
    w_l = [jnp.asarray(wc[f'w{i}']) for i in range(4)]
    g_l = [jnp.asarray(wc[f'g{i}']) for i in range(4)]
    b_l = [jnp.asarray(wc[f'b{i}']) for i in range(4)]
    lff = jnp.asarray(wc['lff_w'][:, :, 0, 0])
    t_w = float(wc['t_w'])
    t_b = float(wc['t_b'])
    c_w1 = jnp.asarray(wc['c_w1'])
    c_b1 = jnp.asarray(wc['c_b1'])
    c_w2 = jnp.asarray(wc['c_w2'])
    c_b2 = jnp.asarray(wc['c_b2'])
    s_w = jnp.asarray(wc['s_w'])
    s_b = float(wc['s_b'])

    def devfn(b0_, b1_, b2_, b3_):
        idx = jax.lax.axis_index('i')
        bidx = idx // 2
        hh = idx % 2
        rows = jnp.arange(R)
        rowmask = jnp.where(hh == 0, (rows >= M), (rows < R - M))
        rowmask = rowmask.astype(jnp.float32)

        bits = jnp.stack([b0_, b1_, b2_, b3_], axis=1)   # [C,T,R,16]
        u = (bits.astype(jnp.int32)[..., None]
             >> jnp.arange(7, -1, -1, dtype=jnp.int32)) & 1
        sx = u.reshape(C, T, R, W).astype(jnp.float32)

        groups = [(sx, M)]
        for i in range(4):
            m_in = 5 - i
            cat = [s[:, :, (ms - m_in):(ms - m_in) + 64 + 2 * m_in, :]
                   for (s, ms) in groups]
            sin = jnp.concatenate(cat, axis=0) if len(cat) > 1 else cat[0]
            y = _conv3(sin, w_l[i])[:, :, 1:-1, :]       # [GR,T,rows,W] f32
            my = m_in - 1
            yint = y[:, :, my:my + 64, :]
            s1 = jnp.sum(yint, axis=(1, 2, 3))
            s2 = jnp.sum(yint * yint, axis=(1, 2, 3))
            st = jax.lax.psum(jnp.concatenate([s1, s2]), 'i')
            cnt = float(T * B * H * W)
            mean = st[:GR] / cnt
            var = st[GR:] / cnt - mean * mean
            scale = g_l[i].astype(jnp.float32) * jax.lax.rsqrt(var + EPS)
            shift = b_l[i] - mean * scale
            ybn = y * scale[:, None, None, None] + shift[:, None, None, None]
            rm = rowmask[M - my: M - my + 64 + 2 * my]
            ybn = ybn * rm[None, None, :, None]
            groups.append((_lif4(ybn), my))

        cat = [s[:, :, (ms - 1):(ms - 1) + 66, :] for (s, ms) in groups]
        sfin = jnp.concatenate(cat, axis=0)              # [160,T,66,W]
        out = jnp.einsum('oi,ithw->othw', lff, sfin,
                         preferred_element_type=jnp.float32)  # [64,T,66,W]

        oint = out[:, :, 1:65, :]
        tot = jnp.sum(oint)
        csum = jnp.sum(oint, axis=(1, 2, 3))             # [C]
        vec = jnp.concatenate([tot[None], csum])
        scat = jnp.where((jnp.arange(B) == bidx)[:, None], vec[None, :], 0.0)
        allb = jax.lax.psum(scat, 'i')
        mine = allb[bidx]
        temp = jax.nn.sigmoid(t_w * mine[0] / float(C * T * H * W) + t_b)
        pooled = temp * mine[1:] / float(T * H * W)
        hid = jax.nn.relu(c_w1 @ pooled + c_b1)
        ca = jax.nn.sigmoid(c_w2 @ hid + c_b2)

        xc = out * (temp * ca)[:, None, None, None]
        rm1 = rowmask[M - 1: M - 1 + 66]
        xc = xc * rm1[None, None, :, None]
        sp = jnp.mean(xc, axis=0, keepdims=True)         # [1,T,66,W]
        sa = _conv3(sp, s_w)[:, :, 1:65, :] + s_b
        sa = jax.nn.sigmoid(sa)                          # [1,T,64,W]
        xs = xc[:, :, 1:65, :] * sa                      # [C,T,64,W]

        q = jnp.clip(jnp.round(xs * (1.0 / QSCALE)), -127, 127)
        return q.astype(jnp.int8)

    return jax.pmap(devfn, axis_name='i', devices=jax.devices()[:ND])


def _get_fn(inputs):
    hsh = hashlib.md5()
    for n in _WNAMES:
        hsh.update(np.ascontiguousarray(inputs[n]).tobytes())
    key = hsh.hexdigest()
    if _cache['key'] != key:
        wc = {n: np.asarray(inputs[n], np.float32) for n in _WNAMES}
        _cache['fn'] = _make_fn(wc)
        _cache['key'] = key
    return _cache['fn']


def kernel(**inputs):
    t00 = time.time()
    x = np.asarray(inputs['x'], np.float32)
    fn = _get_fn(inputs)
    devs = jax.devices()[:ND]

    v = np.zeros((B, C, H, W), np.float32)
    put_pool = ThreadPoolExecutor(2)
    put_futs = []
    for t in range(T):
        np.multiply(v, 0.5, out=v)
        np.add(v, x[t], out=v)
        s = v >= VTH
        bits = np.packbits(s, axis=-1)                   # [B,C,H,16]
        v[s] = 0.0
        shards = []
        for d in range(ND):
            b, hh = d // 2, d % 2
            g0 = hh * 64 - M
            lo, hi = max(g0, 0), min(g0 + R, H)
            sh = np.zeros((C, R, 16), np.uint8)
            sh[:, lo - g0:hi - g0, :] = bits[b, :, lo:hi, :]
            shards.append(sh)
        put_futs.append(put_pool.submit(jax.device_put_sharded, shards, devs))
    targs = [f.result() for f in put_futs]
    put_pool.shutdown(wait=False)
    t01 = time.time()

    q = fn(*targs)                                       # [8,C,T,64,128] int8
    t02 = time.time()

    res = np.empty_like(x)
    shard_by_dev = {sh.device.id: sh.data for sh in q.addressable_shards}
    dev_ids = [d.id for d in devs]

    def fetch(d):
        return d, np.asarray(shard_by_dev[dev_ids[d]])

    with ThreadPoolExecutor(ND) as ex:
        futs = [ex.submit(fetch, d) for d in range(ND)]
        for fu in as_completed(futs):
            d, arr = fu.result()
            qd = arr[0] if arr.ndim == 5 else arr        # [C,T,64,128]
            b, hh = d // 2, d % 2
            deq = np.multiply(qd, np.float32(QSCALE), dtype=np.float32)
            sl = np.s_[:, b, :, hh * 64:(hh + 1) * 64, :]
            np.add(deq.transpose(1, 0, 2, 3), x[sl], out=res[sl])
    t03 = time.time()
    if _PROF:
        print(f'[kprof] lif+upload {1e3*(t01-t00):.0f} ms | pmap dispatch '
              f'{1e3*(t02-t01):.0f} ms | fetch+deq {1e3*(t03-t02):.0f} ms')
    return res
